# revision 29
# baseline (speedup 1.0000x reference)
"""Self-contained Trainium2 Bass kernel for GQA causal self-attention.

Problem: x[2,2048,4096] @ wq/wk/wv (32 q-heads, 8 kv-heads, head_dim 128),
rope (precomputed freqs), causal softmax, GQA attention, wo projection.

Sharding: tensor-parallel across heads over 8 NeuronCores -- core g gets
kv-head g and q-heads 4g..4g+3 (wq/wk/wv column-sharded, wo row-sharded).
Each core computes a partial output projection; the host sums the 8
partials and transposes back (wo is row-parallel so partials just add).

All matmul operands are fp16 (full 1-cycle/row PE rate, 2-byte DMA and
SBUF footprint, 2x DVE throughput); PSUM accumulation stays f32.  Q, K,
V and the attention outputs stay SBUF-resident between phases (no DRAM
spills).  Softmax runs unnormalized with exp(s - 6) so fp16 partial sums
cannot overflow; the bias cancels in the final normalization.

Phase 2 is Act-engine bound: 160 exps of [128,<=1024] at ~1.05us each
(PSUM limits score staging to pair granularity, f32 only on TRN2).  The
pipeline keeps the PE strictly under the Act rate so the exp chain never
stalls: AV matmuls run one pair late; the LAST pair's AV matmuls are
carried past the next q-tile's first score+exp (only its DVE acc-add is
emitted at the tile tail), so the exp chain also streams across q-tile
boundaries; q-tiles iterate descending so head switches land on the
PE-light mask-free tile; mask matmuls are clamped to the 128-wide
diagonal block (the -4096 pattern is zero beyond it); diagonal-pair
exps trim their leading fully-masked columns; the denominator init copy
folds into the first accumulate for jq>0.  Phase-3 output tiles are
interleaved into the batch-1 loop as 2-matmul halves, one per pair
iteration (two on the lighter diagonal iterations), filling the PE slack
under the exp chain.

Phase-1 t=0 weight DMAs are spread across the scalar/gpsimd queues so
the x stream on sync doesn't contend; all small constant DMAs ride
gpsimd so the scalar (Act) sequencer only carries wq + the PSUM-freeing
epilogue copies, which are ordered to match the next tile's accumulator
acquisition order.
"""
import numpy as np
import concourse.bacc as bacc
import concourse.mybir as mybir
import concourse.tile as tile

F32 = mybir.dt.float32
F16 = mybir.dt.float16
AF = mybir.ActivationFunctionType
OP = mybir.AluOpType

P = 128
B, S, D = 2, 2048, 4096
T = B * S            # 4096 tokens
HD = 128             # head dim
NQ = 4               # q heads per core
DC = D // P          # 32 contraction chunks
NT = 512             # free-dim tile
TT = T // NT         # 8 token tiles
SKC = S // P         # 16 s_k chunks per batch
SQT = S // NT        # 4 s_q tiles per batch
KG = 8               # k-chunk groups (4 chunks each) in phase 1
SCALE = 1.0 / float(np.sqrt(HD))
EBIAS = -6.0         # exp(s*SCALE + EBIAS): keeps fp16 sums in range


def build():
    nc = bacc.Bacc("TRN2", target_bir_lowering=False)
    # pre-shuffled inputs (see host_inputs)
    xh = nc.dram_tensor("xh", [TT, KG, P, 4, NT], F16, kind="ExternalInput")
    wqh = nc.dram_tensor("wqh", [P, DC, NQ * HD], F16, kind="ExternalInput")
    wkh = nc.dram_tensor("wkh", [P, DC, HD], F16, kind="ExternalInput")
    wvh = nc.dram_tensor("wvh", [P, DC, HD], F16, kind="ExternalInput")
    woh = nc.dram_tensor("woh", [TT, P, NQ, NT], F16, kind="ExternalInput")
    cosE = nc.dram_tensor("cosE", [P, T], F16, kind="ExternalInput")
    sinE = nc.dram_tensor("sinE", [P, T], F16, kind="ExternalInput")
    ident = nc.dram_tensor("ident", [P, P], F16, kind="ExternalInput")
    ones = nc.dram_tensor("ones", [P, P], F16, kind="ExternalInput")
    mask01 = nc.dram_tensor("mask01", [P, NT // P, NT], F16,
                            kind="ExternalInput")
    outT = nc.dram_tensor("outT", [D, T], F16, kind="ExternalOutput")

    with tile.TileContext(nc) as tc:
        with tc.tile_pool(name="res", bufs=1) as res:
            kT_res = res.tile([P, T], F16)            # roped K^T, resident
            v_res = res.tile([P, DC, HD], F16)        # V natural, resident
            qT_res = res.tile([P, NQ, T], F16)        # roped Q^T, resident
            attn_res = res.tile([P, NQ, T], F16)      # normalized attn^T
            wo_sb = res.tile([P, TT, NQ, NT], F16)
            ones_sb = res.tile([P, P], F16)
            ident_sb = res.tile([P, P], F16)
            mask_sb = res.tile([P, NT // P, NT], F16)
            ebias_sb = res.tile([P, 1], F32)
            nc.vector.memset(ebias_sb[:], EBIAS)

            # ---------------- phase 1: projections + rope ----------------
            with (
                tc.tile_pool(name="c1", bufs=1) as c1,
                tc.tile_pool(name="wgt", bufs=1) as wgt,
                tc.tile_pool(name="xs", bufs=3) as xs,
                tc.tile_pool(name="epi", bufs=2) as epi,
                tc.tile_pool(name="ps_acc", bufs=7, space="PSUM") as ps_acc,
                tc.tile_pool(name="ps_misc", bufs=1, space="PSUM") as ps_misc,
            ):
                wq_sb = c1.tile([P, DC, NQ * HD], F16)
                wk_sb = wgt.tile([P, DC, HD], F16)
                wv_sb = wgt.tile([P, DC, HD], F16)

                for tt in range(TT):
                    tsl = slice(tt * NT, (tt + 1) * NT)
                    cos_t = xs.tile([P, NT], F16, tag="cos", bufs=2)
                    sin_t = xs.tile([P, NT], F16, tag="sin", bufs=2)
                    if tt > 0:
                        nc.gpsimd.dma_start(cos_t[:], cosE[:, tsl])
                        nc.gpsimd.dma_start(sin_t[:], sinE[:, tsl])

                    accs = [ps_acc.tile([P, NT], F32, tag="acc",
                                        name=f"acc{tt}_{oc}")
                            for oc in range(6)]
                    for kg in range(KG):
                        xt = xs.tile([P, 4, NT], F16, tag="xt", bufs=6)
                        if tt == 0 and kg == 0:
                            # fine-grained first transfers so matmul 0 can
                            # start as soon as the k=0 slices land
                            for kc4 in range(4):
                                k4 = slice(kc4, kc4 + 1)
                                nc.scalar.dma_start(wq_sb[:, k4, :],
                                                    wqh[:, k4, :])
                                nc.sync.dma_start(xt[:, k4, :],
                                                  xh[tt, kg, :, k4, :])
                                nc.gpsimd.dma_start(wk_sb[:, k4, :],
                                                    wkh[:, k4, :])
                                nc.gpsimd.dma_start(wv_sb[:, k4, :],
                                                    wvh[:, k4, :])
                        else:
                            if tt == 0:
                                ksl = slice(kg * 4, (kg + 1) * 4)
                                nc.scalar.dma_start(wq_sb[:, ksl, :],
                                                    wqh[:, ksl, :])
                                nc.gpsimd.dma_start(wk_sb[:, ksl, :],
                                                    wkh[:, ksl, :])
                                nc.gpsimd.dma_start(wv_sb[:, ksl, :],
                                                    wvh[:, ksl, :])
                            nc.sync.dma_start(xt[:], xh[tt, kg, :, :, :])
                        for kc in range(4):
                            k = kg * 4 + kc
                            for oc in range(6):
                                if oc < 4:
                                    lhsT = wq_sb[:, k, oc * P:(oc + 1) * P]
                                elif oc == 4:
                                    lhsT = wk_sb[:, k, :]
                                else:
                                    lhsT = wv_sb[:, k, :]
                                nc.tensor.matmul(accs[oc][:], lhsT=lhsT,
                                                 rhs=xt[:, kc, :],
                                                 start=(k == 0),
                                                 stop=(k == DC - 1))

                    if tt == 0:
                        nc.gpsimd.dma_start(cos_t[:], cosE[:, tsl])
                        nc.gpsimd.dma_start(sin_t[:], sinE[:, tsl])
                        nc.gpsimd.dma_start(ident_sb[:], ident[:, :])
                        nc.gpsimd.dma_start(ones_sb[:], ones[:, :])
                        nc.gpsimd.dma_start(mask_sb[:], mask01[:, :, :])

                    # V epilogue first so the transposes reach the
                    # tensor queue immediately after the projections.
                    # At tt=0 the scalar queue is still issuing wq DMA
                    # descriptors, so the copy rides the vector engine.
                    vsb = epi.tile([P, NT], F16, tag="sbr", bufs=5)
                    if tt == 0:
                        nc.vector.tensor_scalar_mul(vsb[:], accs[5][:], 1.0)
                    else:
                        nc.scalar.copy(vsb[:], accs[5][:])
                    # two alternating 128-col regions inside the misc bank
                    # (and alternating copy engines) so transpose c never
                    # waits on the PSUM-freeing copy of transpose c-1
                    vt2 = ps_misc.tile([P, NT], F16, tag="misc",
                                       name="vt_ps")
                    for c in range(NT // P):
                        reg = vt2[:, (c % 2) * P:(c % 2 + 1) * P]
                        nc.tensor.transpose(reg, vsb[:, c * P:(c + 1) * P],
                                            ident_sb[:])
                        if c % 2 == 0:
                            nc.scalar.copy(v_res[:, 4 * tt + c, :], reg)
                        else:
                            nc.vector.tensor_scalar_mul(
                                v_res[:, 4 * tt + c, :], reg, 1.0)

                    # epilogue pass 1: free the Q/K PSUM accumulators in
                    # the order the next tile's matmuls re-acquire them
                    # (bufs=7 rotation: Q0..Q3 then K; vector-first parity
                    # since the scalar queue is still busy with vsb), and
                    # launch the rope pair-swap DMAs.
                    sbs, sws = [], []
                    for ei, oc in enumerate((0, 1, 2, 3, 4)):
                        sb_r = epi.tile([P, NT], F16, tag="sbr", bufs=5)
                        if ei % 2 == 0:
                            nc.vector.tensor_scalar_mul(sb_r[:], accs[oc][:],
                                                        1.0)
                        else:
                            nc.scalar.copy(sb_r[:], accs[oc][:])
                        sw = epi.tile([P, NT], F16, tag="sw", bufs=5)
                        nc.gpsimd.dma_start(sw[1::2, :], sb_r[0::2, :])
                        nc.gpsimd.dma_start(sw[0::2, :], sb_r[1::2, :])
                        sbs.append(sb_r)
                        sws.append(sw)

                    # epilogue pass 2: all-fp16 DVE rope math
                    for i, oc in enumerate((0, 1, 2, 3, 4)):
                        t1 = epi.tile([P, NT], F16, tag="t1", bufs=2)
                        nc.vector.tensor_tensor(t1[:], sbs[i][:], cos_t[:],
                                                op=OP.mult)
                        t2 = epi.tile([P, NT], F16, tag="t2")
                        nc.vector.tensor_tensor(t2[:], sws[i][:], sin_t[:],
                                                op=OP.mult)
                        if oc < 4:
                            nc.vector.tensor_tensor(qT_res[:, oc, tsl],
                                                    t1[:], t2[:], op=OP.add)
                        else:
                            nc.vector.tensor_tensor(kT_res[:, tsl],
                                                    t1[:], t2[:], op=OP.add)

            # ---------------- phase 2: attention ----------------
            with (
                tc.tile_pool(name="pts", bufs=3) as pts,
                tc.tile_pool(name="accp", bufs=3) as accp,
                tc.tile_pool(name="ep2", bufs=3) as ep2,
                tc.tile_pool(name="op3", bufs=4) as op3,
                tc.tile_pool(name="ps_st", bufs=2, space="PSUM") as ps_st,
                tc.tile_pool(name="ps_att", bufs=2, space="PSUM") as ps_att,
                tc.tile_pool(name="ps_den", bufs=2, space="PSUM") as ps_den,
            ):
                # softmax denominator + normalization for a finished q-tile;
                # called one q-tile late so it never stalls the tensor queue
                def emit_den(p):
                    acc_, att_, h_, qsl_ = p
                    den_ps = ps_den.tile([P, NT], F32, tag="den")
                    nc.tensor.matmul(den_ps[:], lhsT=ones_sb[:],
                                     rhs=acc_[:, 0, :], start=True, stop=False)
                    nc.tensor.matmul(den_ps[:], lhsT=ones_sb[:],
                                     rhs=acc_[:, 1, :], start=False, stop=True)
                    rc = ep2.tile([P, NT], F32, tag="rc")
                    nc.vector.reciprocal_approx_fast(rc[:], den_ps[:])
                    nc.vector.tensor_tensor(attn_res[:, h_, qsl_],
                                            att_[:], rc[:], op=OP.mult)

                # AV matmuls for a finished pair.  pv carries its whole
                # context so the last pair of a q-tile can be emitted
                # AFTER the next q-tile's first score+exp (the exp chain
                # then streams across the boundary without a bubble).
                def emit_av_mms(pv):
                    pt_, cs_, ip_, b_, nk_, att_ = pv[:6]
                    for half in range(2):
                        ik = 2 * ip_ + half
                        nc.tensor.matmul(
                            att_[:, cs_[half]:],
                            lhsT=v_res[:, 16 * b_ + ik, :],
                            rhs=pt_[:, half, cs_[half]:],
                            start=(ik == 0), stop=(ik == nk_ - 1))

                # denominator partials on the vector engine.  pt columns
                # below cs_ hold exp(stale-PSUM) garbage and must never
                # reach acc.  For jq>0 the first two pairs are untrimmed,
                # so their adds fold into one op (no init copy).
                def emit_acc(pv):
                    pt_, cs_, ip_, b_, nk_, att_, acc_, jq_, pt_prev_ = pv
                    if ip_ == 0:
                        if jq_ == 0:
                            nc.vector.tensor_scalar_mul(acc_[:], pt_[:], 1.0)
                    elif ip_ == 1 and jq_ > 0:
                        nc.vector.tensor_tensor(acc_[:], pt_prev_[:], pt_[:],
                                                op=OP.add)
                    elif cs_[0] == 0 and cs_[1] == 0:
                        nc.vector.tensor_tensor(acc_[:], acc_[:], pt_[:],
                                                op=OP.add)
                    else:
                        for half in range(2):
                            c0 = cs_[half]
                            nc.vector.tensor_tensor(
                                acc_[:, half, c0:], acc_[:, half, c0:],
                                pt_[:, half, c0:], op=OP.add)

                # one phase-3 output tile (oc = 128-row block of the wo
                # output, jt = 512-token column tile), interleaved mid-tile
                # into phase 2's batch-1 loop where the PE has slack under
                # the Act-bound exp chain.  Shares the ps_den banks.
                p3_done = set()
                p3_half = [None]   # in-flight half-emitted p3 item

                def emit_p3_half():
                    # emit half an interleaved phase-3 item (2 of its 4
                    # chained matmuls) so each insertion fits the ~0.2us
                    # per-pair PE slack under the Act-bound exp chain
                    if p3_half[0] is None:
                        if p3_i[0] >= len(p3_queue):
                            return
                        jt, oc = p3_queue[p3_i[0]]
                        p3_i[0] += 1
                        jsl = slice(jt * NT, (jt + 1) * NT)
                        o_ps = ps_den.tile([P, NT], F32, tag="den",
                                           name="o_ps")
                        wj, wn = oc // 4, (oc % 4) * P
                        for dc in (0, 1):
                            nc.tensor.matmul(
                                o_ps[:],
                                lhsT=wo_sb[:, wj, dc, wn:wn + P],
                                rhs=attn_res[:, dc, jsl],
                                start=(dc == 0), stop=False)
                        p3_half[0] = (jt, oc, o_ps)
                    else:
                        jt, oc, o_ps = p3_half[0]
                        p3_half[0] = None
                        jsl = slice(jt * NT, (jt + 1) * NT)
                        wj, wn = oc // 4, (oc % 4) * P
                        for dc in (2, 3):
                            nc.tensor.matmul(
                                o_ps[:],
                                lhsT=wo_sb[:, wj, dc, wn:wn + P],
                                rhs=attn_res[:, dc, jsl],
                                start=False, stop=(dc == 3))
                        osb = op3.tile([P, NT], F16, tag="ot")
                        nc.vector.tensor_scalar_mul(osb[:], o_ps[:], 1.0)
                        nc.sync.dma_start(outT[oc * P:(oc + 1) * P, jsl],
                                          osb[:])
                        p3_done.add((jt, oc))

                p3_queue = [(jt, oc) for oc in range(D // P)
                            for jt in range(SQT)]
                p3_i = [0]

                pending = None   # q-tile awaiting denominator/normalize
                carry = None     # last pair of previous q-tile (AV mms)
                for b in range(B):
                    for h in range(NQ):
                        # wo prefetch: both chunks during b=0
                        if b == 0:
                            for wj in (h, 4 + h):
                                nc.gpsimd.dma_start(wo_sb[:, wj, :, :],
                                                    woh[wj, :, :, :])
                        for jq in reversed(range(SQT)):
                            nk = 4 * (jq + 1)
                            att_ps = ps_att.tile([P, NT], F32, tag="attn")
                            acc = accp.tile([P, 2, NT], F16, tag="acc")
                            qsl = slice(b * S + jq * NT, b * S + (jq + 1) * NT)

                            prev = None
                            pt0 = None
                            for ip in range(nk // 2):   # ik pairs
                                st = ps_st.tile([P, 2, NT], F32, tag="st")
                                # first valid q column per half (causal trim);
                                # ip 0 stays full so the acc init is clean
                                cs = []
                                for half in range(2):
                                    ik = 2 * ip + half
                                    r = ik - 4 * jq
                                    c0 = 128 * r if (r >= 1 and ip > 0) else 0
                                    cs.append(c0)
                                    nc.tensor.matmul(
                                        st[:, half, c0:],
                                        lhsT=kT_res[:, b * S + ik * P:
                                                    b * S + (ik + 1) * P],
                                        rhs=qT_res[:, h, qsl][:, c0:],
                                        start=True, stop=(r < 0))
                                    if r >= 0:
                                        # causal mask: add -4096 pattern.
                                        # mask01[:, r, n] == 0 for all
                                        # n >= 128(r+1), so clamp the mask
                                        # matmul to the diagonal block
                                        # (columns beyond it keep the score
                                        # matmul's value unchanged)
                                        m_end = 128 * r + 128
                                        nc.tensor.matmul(
                                            st[:, half, c0:m_end],
                                            lhsT=ident_sb[:],
                                            rhs=mask_sb[:, r, c0:m_end],
                                            start=False, stop=True)
                                pt = pts.tile([P, 2, NT], F16, tag="pt")
                                # the (r2,r3) diagonal pair of jq>=1 tiles:
                                # exp + acc trimmed to the first valid column
                                r0 = 2 * ip - 4 * jq
                                t0 = 256 if r0 == 2 else 0
                                nc.scalar.activation(pt[:, :, t0:],
                                                     st[:, :, t0:],
                                                     AF.Exp, scale=SCALE,
                                                     bias=ebias_sb[:])
                                if ip == 0:
                                    if carry is not None:
                                        emit_av_mms(carry)
                                        carry = None
                                else:
                                    emit_av_mms(prev)
                                    emit_acc(prev)
                                    if ip == 1 and pending is not None:
                                        emit_den(pending)
                                        pending = None
                                    if b == 1 and ip >= 2:
                                        emit_p3_half()
                                        if r0 >= 0:
                                            # diagonal iterations are
                                            # PE-light (clamped masks):
                                            # room for a second half
                                            emit_p3_half()
                                prev = (pt, cs, ip, b, nk, att_ps, acc, jq,
                                        pt0)
                                pt0 = pt
                            # tail: the DVE acc-add runs now (so the
                            # denominator is complete), but the AV matmuls
                            # are deferred past the next tile's first
                            # score+exp
                            emit_acc(prev)
                            carry = prev
                            pending = (acc, att_ps, h, qsl)
                if carry is not None:
                    emit_av_mms(carry)
                if pending is not None:
                    emit_den(pending)
                if p3_half[0] is not None:
                    emit_p3_half()

            # ---------------- phase 3: output projection ----------------
            with (
                tc.tile_pool(name="outp", bufs=8) as outp,
                tc.tile_pool(name="ps_o", bufs=8, space="PSUM") as ps_o,
            ):
                items = [(jt, oc) for jt in range(TT)
                         for oc in range(D // P) if (jt, oc) not in p3_done]
                for n_it, (jt, oc) in enumerate(items):
                    jsl = slice(jt * NT, (jt + 1) * NT)
                    o_ps = ps_o.tile([P, NT], F32, tag="o")
                    wj, wn = oc // 4, (oc % 4) * P
                    for dc in range(NQ):
                        nc.tensor.matmul(
                            o_ps[:],
                            lhsT=wo_sb[:, wj, dc, wn:wn + P],
                            rhs=attn_res[:, dc, jsl],
                            start=(dc == 0), stop=(dc == NQ - 1))
                    osb = outp.tile([P, NT], F16, tag="ot")
                    if oc % 2 == 0:
                        nc.scalar.copy(osb[:], o_ps[:])
                    else:
                        nc.vector.tensor_scalar_mul(osb[:], o_ps[:], 1.0)
                    nc.sync.dma_start(
                        outT[oc * P:(oc + 1) * P, jsl], osb[:])

    nc.compile()
    return nc


def host_inputs(x, wq, wk, wv, wo, freqs_cos, freqs_sin):
    """Build the 8 per-core input maps from full inputs (pre-shuffled)."""
    x2 = np.asarray(x, dtype=np.float32).reshape(T, D)
    # xh[tt, kg, p, kc, n] = x2[tt*NT + n, kg*512 + kc*128 + p]
    xh = np.ascontiguousarray(
        x2.reshape(TT, NT, KG, 4, P).transpose(0, 2, 4, 3, 1)).astype(
        np.float16)
    fc = np.asarray(freqs_cos, dtype=np.float32)
    fs = np.asarray(freqs_sin, dtype=np.float32)
    cc = np.repeat(fc.T, 2, axis=0)                         # [128, S]
    ss = np.repeat(fs.T, 2, axis=0)
    sgn = np.ones((P, 1), np.float32)
    sgn[0::2, 0] = -1.0
    cosE = np.ascontiguousarray(np.tile(cc, (1, B)).astype(np.float16))
    sinE = np.ascontiguousarray(np.tile(ss * sgn, (1, B)).astype(np.float16))
    ident_np = np.eye(P, dtype=np.float16)
    ones_np = np.ones((P, P), np.float16)
    mk = np.zeros((P, NT // P, NT), np.float16)
    for r in range(NT // P):
        for p in range(P):
            mk[p, r, :] = np.where(np.arange(NT) >= 128 * r + p, 0.0, -4096.0)

    wq_f = np.asarray(wq, dtype=np.float32)
    wk_f = np.asarray(wk, dtype=np.float32)
    wv_f = np.asarray(wv, dtype=np.float32)
    wo_f = np.asarray(wo, dtype=np.float32)
    in_maps = []
    for g in range(8):
        wq_g = wq_f[:, g * NQ * HD:(g + 1) * NQ * HD]       # [D, 512]
        wk_g = wk_f[:, g * HD:(g + 1) * HD]                 # [D, 128]
        wv_g = wv_f[:, g * HD:(g + 1) * HD]
        wo_g = wo_f[g * NQ * HD:(g + 1) * NQ * HD, :]       # [512, D]
        # [P, DC, M] with element [p, c, m] = w[c*128 + p, m]
        wqh_np = np.ascontiguousarray(
            wq_g.reshape(DC, P, NQ * HD).transpose(1, 0, 2)).astype(np.float16)
        wkh_np = np.ascontiguousarray(
            wk_g.reshape(DC, P, HD).transpose(1, 0, 2)).astype(np.float16)
        wvh_np = np.ascontiguousarray(
            wv_g.reshape(DC, P, HD).transpose(1, 0, 2)).astype(np.float16)
        # woh[j, p, dc, n] = wo_g[dc*128 + p, j*512 + n]
        woh_np = np.ascontiguousarray(
            wo_g.reshape(NQ, P, TT, NT).transpose(2, 1, 0, 3)).astype(
            np.float16)
        in_maps.append({
            "xh": xh, "wqh": wqh_np, "wkh": wkh_np, "wvh": wvh_np,
            "woh": woh_np,
            "cosE": cosE, "sinE": sinE, "ident": ident_np,
            "ones": ones_np, "mask01": mk,
        })
    return in_maps


def combine_outputs(results):
    """Sum per-core partial^T and transpose back to [B, S, D]."""
    acc = results[0]["outT"].astype(np.float32)
    for r in results[1:]:
        acc += r["outT"].astype(np.float32)
    return np.ascontiguousarray(acc.T).reshape(B, S, D).astype(np.float32)


_NC = None


def kernel(x, wq, wk, wv, wo, freqs_cos, freqs_sin):
    """Full-input entry point: shards across 8 cores, runs, gathers."""
    global _NC
    from concourse.bass_utils import run_bass_kernel_spmd
    if _NC is None:
        _NC = build()
    in_maps = host_inputs(x, wq, wk, wv, wo, freqs_cos, freqs_sin)
    res = run_bass_kernel_spmd(_NC, in_maps, core_ids=list(range(8)),
                               trace=False)
    return combine_outputs(res.results)
